# revision 49
# baseline (speedup 1.0000x reference)
"""Trainium2 Bass kernel for a MixEncoderLayer (attention w/ additive cost
matrix bias + FFN), batch 8, seq 1024, d_model 512, 8 heads, d_ff 2048.

Strategy: pure data parallelism — one batch element per NeuronCore, 8 cores,
no collectives.  Inside each core:

  X^T / W^T built via PE transposes; all matmuls in float32r (full-rate
  fp32 streaming on the PE at N=512).

  Attention is computed in "key-major" layout: scores^T[k, q], so softmax
  weights come out in exactly the layout needed as the moving operand of
  the attn@V matmul (no transposes of the 8.4M-element attention matrix).
  The cost-matrix bias is preloaded into PSUM with an identity-matmul
  (PE moves elements ~2.5x faster than DVE), the QK^T matmul accumulates
  on top, and the ACT engine applies exp directly from a wide 4-bank PSUM
  tile (amortizing its 352-cycle fixed overhead).  Softmax skips max
  subtraction (scores are O(+-6), exp is safe in f32); row sums come from
  augmenting V with a ones column ([V_h | 1], M=65) so ctx^T and rowsum^T
  fall out of one PSUM accumulation group; normalization is a reciprocal +
  ones-matmul partition-broadcast multiply on ctx^T (64x1024 per head)
  instead of on the 1024x1024 attention matrix.

Pools are stack-allocated per side; left = long-lived (released at end,
LIFO), right = stage-scoped.
"""

import numpy as np

import concourse.bass as bass
import concourse.mybir as mybir
import concourse.tile as tile
from concourse.masks import make_identity

F32 = mybir.dt.float32
F32R = mybir.dt.float32r
FP8 = mybir.dt.float8e4
BF16 = mybir.dt.bfloat16
AF = mybir.ActivationFunctionType
ALU = mybir.AluOpType
DR = mybir.MatmulPerfMode.DoubleRow

S, Dm, H, DK, DF = 1024, 512, 8, 64, 2048
ST, DT, FT = S // 128, Dm // 128, DF // 128  # 8, 4, 16
NCORES = 8
LN_EPS = 1e-6
INV_SQRT_DK = 0.125  # 1/sqrt(64)

INPUT_SHAPES = {
    "enc_input": (S, Dm),
    "cost_mat": (S, S),
    "wq": (Dm, Dm),
    "wk": (Dm, Dm),
    "wv": (Dm, Dm),
    "fc_w": (Dm, Dm),
    "ln1_g": (Dm,),
    "ln1_b": (Dm,),
    "w1": (DF, Dm),
    "b1": (DF,),
    "w2": (Dm, DF),
    "b2": (Dm,),
    "ln2_g": (Dm,),
    "ln2_b": (Dm,),
}


def _build(tc, io, out_ap):
    nc = tc.nc
    with nc.allow_low_precision(reason="f32r matmul operands; accumulation stays f32 in PSUM"):
        _build_inner(tc, io, out_ap)


def _build_inner(tc, io, out_ap):
    nc = tc.nc
    ev_cnt = [0]

    def evict_copy(dst, src, scale=None, eng="v"):
        """PSUM -> SBUF copy.  eng='v': DVE; eng='s': ACT (used for stage-A
        evictions where the ACT engine is otherwise idle)."""
        ev_cnt[0] += 1
        if eng == "s":
            if scale is None:
                nc.scalar.copy(dst, src)
            else:
                nc.scalar.mul(dst, src, scale)
        elif scale is None:
            nc.vector.tensor_copy(dst, src)
        else:
            nc.vector.tensor_scalar_mul(out=dst, in0=src, scalar1=scale)

    # ---------------- long-lived pools (left stack) ----------------
    singles = tc.alloc_tile_pool(name="singles", bufs=1, side="left")
    # right-stack pools that live A/C -> D (bottom of the right stack)
    p_fcw = tc.alloc_tile_pool(name="p_fcw", bufs=1, side="right")
    p_ctx = tc.alloc_tile_pool(name="p_ctx", bufs=1, side="right")

    ident = singles.tile([128, 128], F32, tag="ident")
    make_identity(nc, ident)
    identR = singles.tile([128, 128], F32R, tag="identR")
    nc.vector.tensor_copy(identR, ident)
    # fp8 [I|0] / [0|I] stationaries: DoubleRow "copy" of one 512-col half of
    # a [128, 2, 512] cost tile into PSUM at 2 cols/cycle (2x the f32r
    # identity-matmul preload).
    idzA = singles.tile([128, 2, 128], FP8, tag="idzA")
    nc.gpsimd.memset(idzA, 0.0)
    nc.vector.tensor_copy(idzA[:, 0, :], ident)
    idzB = singles.tile([128, 2, 128], FP8, tag="idzB")
    nc.gpsimd.memset(idzB, 0.0)
    nc.vector.tensor_copy(idzB[:, 1, :], ident)
    eps_t = singles.tile([128, 1], F32, tag="eps")
    nc.gpsimd.memset(eps_t, LN_EPS)
    ones_f32 = singles.tile([128, 1], F32, tag="ones_f32")
    nc.vector.memset(ones_f32, 1.0)
    ones8_f32 = singles.tile([128, 1], F32, tag="ones8_f32")
    nc.vector.memset(ones8_f32, 8.0)
    c32_f32 = singles.tile([128, 1], F32, tag="c32_f32")
    nc.vector.memset(c32_f32, 32.0)
    # rowsum-broadcast stationary carries the x32 ctx fp8 scale
    ones_t = singles.tile([128, 64], F32R, tag="ones")
    nc.vector.tensor_copy(ones_t, c32_f32.to_broadcast((128, 64)))
    fcrs_t = singles.tile([128, 1], F32, tag="fcrs")
    nc.vector.memset(fcrs_t, 1.0 / (32.0 * 32.0))
    # exp(score - 1.5): keeps exp outputs under fp8e4m3's 448 max for the
    # score tails (cancels exactly in the softmax normalization)
    ebias_t = singles.tile([128, 1], F32, tag="ebias")
    nc.vector.memset(ebias_t, -1.5)

    def layer_norm(src, dst, g_b, b_b, pool, gb_eng=None, stt_eng=None):
        """dst = LN(src) * g + b over free dim (512).  gb_eng picks which
        engine applies g/b: DVE when the result feeds the PE (LN1 -> aoT
        transposes), the idle-but-slow gpsimd when it only feeds DMA (LN2)."""
        gb = gb_eng if gb_eng is not None else nc.gpsimd
        stats = pool.tile([128, 6], F32, tag="ln_stats", bufs=3, name="ln_stats")
        mv = pool.tile([128, 2], F32, tag="ln_mv", bufs=3, name="ln_mv")
        nc.vector.bn_stats(out=stats, in_=src)
        nc.vector.bn_aggr(out=mv, in_=stats)
        istd = pool.tile([128, 1], F32, tag="ln_istd", bufs=3, name="ln_istd")
        nc.scalar.activation(out=istd, in_=mv[:, 1:2], func=AF.Sqrt, bias=eps_t)
        nc.vector.reciprocal(out=istd, in_=istd)
        xn = pool.tile([128, Dm], F32, tag="ln_xn", bufs=2, name="ln_xn")
        (stt_eng if stt_eng is not None else nc.vector).scalar_tensor_tensor(
            out=xn, in0=src, scalar=mv[:, 0:1], in1=istd.to_broadcast((128, Dm)),
            op0=ALU.subtract, op1=ALU.mult)
        gb.tensor_mul(dst, xn, g_b)
        gb.tensor_add(dst, dst, b_b)

    # ================= stage A: loads + transposes =================
    p_x = tc.alloc_tile_pool(name="p_x", bufs=1, side="right")      # A -> D
    p_dtmp = tc.alloc_tile_pool(name="p_dtmp", bufs=2, side="right")  # D scratch
    p_cost = tc.alloc_tile_pool(name="p_cost", bufs=1, side="right")  # A -> C
    p_qkv = tc.alloc_tile_pool(name="p_qkv", bufs=1, side="right")  # B -> C
    p_ab = tc.alloc_tile_pool(name="p_ab", bufs=1, side="right")    # A -> B
    p_stgA = tc.alloc_tile_pool(name="p_stgA", bufs=5, side="right")  # A only
    tps = tc.alloc_tile_pool(name="tps", bufs=4, space="PSUM", side="right")
    bps = tc.alloc_tile_pool(name="bps", bufs=3, space="PSUM", side="right")

    def transpose_quad(psum_pool, psum_tag, dst_wide, srcs, scale=None, eng="v"):
        """Transpose up to 4 [128,128] blocks into one PSUM bank, evict once.
        f32r-mode (1.5 cyc/row vs 2 for f32) when the source tile is f32r."""
        n = len(srcs)
        ps = psum_pool.tile([128, n * 128], F32, tag=psum_tag, name=psum_tag)
        r = srcs[0].dtype == F32R
        idt = identR if r else ident
        for i, s in enumerate(srcs):
            sl = ps[:, i * 128:(i + 1) * 128]
            nc.tensor.transpose(sl.bitcast(F32R) if r else sl, s, idt)
        evict_copy(dst_wide, ps, scale=scale, eng=eng)

    # X + X^T
    xsb = []
    for st in range(ST):
        t = p_x.tile([128, Dm], F32R, tag=f"x{st}", name=f"x{st}")
        nc.sync.dma_start(
            out=t,
            in_=io["enc_input"][st * 128:(st + 1) * 128, :].bitcast(F32R))
        xsb.append(t)
    # X^T, W^T in fp8 with the d-tile PAIR in dim1 (DoubleRow contraction):
    # QKV/fc projections then take 2 DR matmuls instead of 4 f32r ones.
    # Weights are pre-scaled x32 into fp8 normal range (std 0.02 -> 0.64);
    # the x32 is divided back out at each PSUM eviction.
    WS = 32.0
    XT2 = [p_ab.tile([128, 2, S], FP8, tag=f"xt{dp}", name=f"xt{dp}")
           for dp in range(DT // 2)]
    XT = [XT2[d // 2][:, d % 2, :] for d in range(DT)]
    for d in range(DT):
        for g in range(ST // 4):
            transpose_quad(
                tps, "tps", XT[d][:, g * 512:(g + 1) * 512],
                [xsb[g * 4 + i][:, d * 128:(d + 1) * 128] for i in range(4)],
                eng="s")

    def load_transposed(stg_pool, psum_pool, psum_tag, wap, dst_tiles, stg_tag,
                        group=4, scale=None, eng="v", dma="sync"):
        """wap: DRAM [nout, nin]; dst_tiles[k]: [128, nout] covering nin rows."""
        nout, nin = wap.shape
        nit = nout // 128
        dma_eng = nc.sync if dma == "sync" else nc.gpsimd
        for g in range(0, nit, group):
            n = min(group, nit - g)
            stgs = []
            for i in range(n):
                stg = stg_pool.tile([128, nin], F32R, tag=stg_tag, name=stg_tag)
                dma_eng.dma_start(
                    out=stg,
                    in_=wap[(g + i) * 128:(g + i + 1) * 128, :].bitcast(F32R))
                stgs.append(stg)
            for dt_ in range(nin // 128):
                transpose_quad(
                    psum_pool, psum_tag,
                    dst_tiles[dt_][:, g * 128:(g + n) * 128],
                    [stgs[i][:, dt_ * 128:(dt_ + 1) * 128] for i in range(n)],
                    scale=scale, eng=eng)

    wqT2 = [p_ab.tile([128, 2, Dm], FP8, tag=f"wqt{dp}", name=f"wqt{dp}")
            for dp in range(DT // 2)]
    wkT2 = [p_ab.tile([128, 2, Dm], FP8, tag=f"wkt{dp}", name=f"wkt{dp}")
            for dp in range(DT // 2)]
    wvT2 = [p_ab.tile([128, 2, Dm], FP8, tag=f"wvt{dp}", name=f"wvt{dp}")
            for dp in range(DT // 2)]
    fcwT2 = [p_fcw.tile([128, 2, Dm], FP8, tag=f"fcwt{dp}", name=f"fcwt{dp}")
             for dp in range(DT // 2)]
    wqT = [wqT2[d // 2][:, d % 2, :] for d in range(DT)]
    wkT = [wkT2[d // 2][:, d % 2, :] for d in range(DT)]
    wvT = [wvT2[d // 2][:, d % 2, :] for d in range(DT)]
    fcwT = [fcwT2[d // 2][:, d % 2, :] for d in range(DT)]
    load_transposed(p_stgA, tps, "tps", io["wq"], wqT, "stg512", scale=WS, eng="s")
    load_transposed(p_stgA, tps, "tps", io["wk"], wkT, "stg512", scale=WS, eng="s")
    load_transposed(p_stgA, tps, "tps", io["wv"], wvT, "stg512", scale=WS, eng="s", dma="pool")

    # small constants on the Pool-engine DMA queue so they don't delay the
    # big input loads on the SP queue or the ACT-engine evictions
    def bcast_row(name, src1d):  # [Dm] dram -> [128, Dm] sbuf (partition bcast)
        t = singles.tile([128, Dm], F32, tag=name, name=name)
        nc.scalar.dma_start(out=t, in_=src1d[None, :].to_broadcast((128, Dm)))
        return t

    ln1g_b = bcast_row("ln1g_b", io["ln1_g"])
    ln1b_b = bcast_row("ln1b_b", io["ln1_b"])
    ln2g_b = bcast_row("ln2g_b", io["ln2_g"])
    ln2b_b = bcast_row("ln2b_b", io["ln2_b"])
    b2_b = bcast_row("b2_b", io["b2"])
    b1_t = []
    for jt in range(FT):
        t = singles.tile([128, 1], F32, tag=f"b1_{jt}", name=f"b1_{jt}")
        nc.scalar.dma_start(out=t, in_=io["b1"][jt * 128:(jt + 1) * 128][:, None])
        b1_t.append(t)

    # cost^T in fp8 as [128, 2, 512] (the two 512-col q-halves are the
    # DoubleRow k-pair slots; idzA/idzB select which half lands in PSUM).
    costT = [p_cost.tile([128, 2, 512], FP8, tag=f"ct{k}", name=f"ct{k}")
             for k in range(ST)]
    costT_2d = [ct[:, :, :].rearrange("p a b -> p (a b)") for ct in costT]
    load_transposed(p_stgA, tps, "tps", io["cost_mat"], costT_2d, "stg1024", eng="s", dma="pool")
    load_transposed(p_stgA, tps, "tps", io["fc_w"], fcwT, "stg512", scale=WS, eng="s", dma="pool")

    # ================= stage B: QKV projections =================
    QT = [p_qkv.tile([128, S], F32R, tag=f"qt{i}", name=f"qt{i}") for i in range(DT)]
    KT = [p_qkv.tile([128, S], F32R, tag=f"kt{i}", name=f"kt{i}") for i in range(DT)]
    # V (x8, fp8) + ones(=8) column, t-pairs in dim1 for the DoubleRow ctx
    # matmul; per-head stride 80B keeps every AP step 16B-aligned.  The x8
    # scale cancels in the rowsum normalization.
    VP = 80
    vaug = [p_qkv.tile([128, 2, H, VP], FP8, tag=f"va{t2}", name=f"va{t2}")
            for t2 in range(ST // 2)]

    for it in range(DT):
        for c in range(2):
            ps = bps.tile([128, 512], F32, tag="bps", name="bps")
            for p in range(DT // 2):
                nc.tensor.matmul(ps, wqT2[p][:, :, it * 128:(it + 1) * 128],
                                 XT2[p][:, :, c * 512:(c + 1) * 512],
                                 start=(p == 0), stop=(p == DT // 2 - 1),
                                 perf_mode=DR)
            # fold 1/sqrt(dk) and the 1/32 weight-scale into Q
            nc.vector.tensor_scalar_mul(
                out=QT[it][:, c * 512:(c + 1) * 512], in0=ps,
                scalar1=INV_SQRT_DK / WS)
    for it in range(DT):
        for c in range(2):
            ps = bps.tile([128, 512], F32, tag="bps", name="bps")
            for p in range(DT // 2):
                nc.tensor.matmul(ps, wkT2[p][:, :, it * 128:(it + 1) * 128],
                                 XT2[p][:, :, c * 512:(c + 1) * 512],
                                 start=(p == 0), stop=(p == DT // 2 - 1),
                                 perf_mode=DR)
            nc.scalar.mul(KT[it][:, c * 512:(c + 1) * 512], ps, 1.0 / WS)
    for st in range(ST):
        t2, tj = divmod(st, 2)
        nc.vector.tensor_copy(
            out=vaug[t2][:, tj, :, DK:DK + 1].rearrange("p h o -> p (h o)"),
            in_=ones8_f32.to_broadcast((128, H)))
        ps = bps.tile([128, 512], F32, tag="bps", name="bps")
        for p in range(DT // 2):
            nc.tensor.matmul(ps, XT2[p][:, :, st * 128:(st + 1) * 128], wvT2[p],
                             start=(p == 0), stop=(p == DT // 2 - 1),
                             perf_mode=DR)
        nc.vector.tensor_scalar_mul(
            out=vaug[t2][:, tj, :, 0:DK],
            in0=ps.rearrange("p (h e) -> p h e", h=H), scalar1=8.0 / WS)

    p_stgA.release()
    p_ab.release()
    bps.release()
    tps.release()

    # ================= stage C: attention (key-major) =================
    # ctx^T in fp8 (x32) with d-tile pairs in dim1 for the DR fc matmul
    ctxT2 = [p_ctx.tile([128, 2, S], FP8, tag=f"cx{dp}", name=f"cx{dp}")
             for dp in range(DT // 2)]
    p_c = tc.alloc_tile_pool(name="p_c", bufs=2, side="right")
    scpsW = tc.alloc_tile_pool(name="scpsW", bufs=2, space="PSUM", side="right")
    ctxps = tc.alloc_tile_pool(name="ctxps", bufs=3, space="PSUM", side="right")

    # w1 + w2 load+transpose traced mid-C: fills PE gaps during the
    # ACT-bound attention phase (left-side space, no deps on stage-C pools).
    p_w1 = tc.alloc_tile_pool(name="p_w1", bufs=1, side="left")
    p_stgW1 = tc.alloc_tile_pool(name="p_stgW1", bufs=5, side="left")
    tpsW = tc.alloc_tile_pool(name="tpsW", bufs=1, space="PSUM", side="left")
    w1T = [p_w1.tile([128, DF], BF16, tag=f"w1t{d}", name=f"w1t{d}")
           for d in range(DT)]
    load_transposed(p_stgW1, tpsW, "tpsW", io["w1"], w1T, "stgw1")
    p_stgW1.release()
    p_w2 = tc.alloc_tile_pool(name="p_w2", bufs=1, side="left")
    p_stgW2 = tc.alloc_tile_pool(name="p_stgW2", bufs=3, side="left")
    w2T = [p_w2.tile([128, Dm], BF16, tag=f"w2t{j}", name=f"w2t{j}")
           for j in range(FT)]
    load_transposed(p_stgW2, tpsW, "tpsW", io["w2"], w2T, "stgw2", group=2,
                    dma="pool")
    p_stgW2.release()
    tpsW.release()
    # 1-bank PSUM pool + stage-D SBUF, so the c2=0 half of fc/LN1/aoT can
    # run woven into attention c=1 (all 7 right-side banks are then held by
    # the attention pools).
    dps = tc.alloc_tile_pool(name="dps", bufs=1, space="PSUM", side="left")
    p_d = tc.alloc_tile_pool(name="p_d", bufs=1, side="left")  # D -> E
    attn_out = [p_d.tile([128, Dm], F32, tag=f"ao{st}", name=f"ao{st}")
                for st in range(ST)]
    aoT = [p_d.tile([128, S], BF16, tag=f"aot{d}", name=f"aot{d}") for d in range(DT)]

    def d_stage(st, psum_pool):
        """fc + residual + LN1 for one token tile."""
        ps = psum_pool.tile([128, 512], F32, tag="fcps", name="fcps")
        for p in range(DT // 2):
            nc.tensor.matmul(ps, ctxT2[p][:, :, st * 128:(st + 1) * 128],
                             fcwT2[p], start=(p == 0), stop=(p == DT // 2 - 1),
                             perf_mode=DR)
        a = p_dtmp.tile([128, Dm], F32, tag="attnin", name="attnin")
        # undo the x32 ctx and x32 fc_w fp8 scales, then add the residual
        nc.vector.scalar_tensor_tensor(
            out=a, in0=ps, scalar=fcrs_t, in1=xsb[st],
            op0=ALU.mult, op1=ALU.add)
        layer_norm(a, attn_out[st], ln1g_b, ln1b_b, p_dtmp)

    def aot_quads(g, psum_pool):
        for d in range(DT):
            transpose_quad(
                psum_pool, "fcps", aoT[d][:, g * 512:(g + 1) * 512],
                [attn_out[g * 4 + i][:, d * 128:(d + 1) * 128]
                 for i in range(4)])

    # c outer so the fc matmuls of stage D (which need all heads but only one
    # 512-token half) can start when attention is half done.
    NT2 = ST // 2
    for c in range(2):
        idz = idzA if c == 0 else idzB
        for hp in range(H // 2):
            cps = [ctxps.tile([DK + 1, 512], F32, tag="ctxps", name="ctxps")
                   for _ in range(2)]
            scs = [None] * NT2

            def ctx_mms(t2):
                for hi in range(2):
                    nc.tensor.matmul(
                        cps[hi], vaug[t2][:, :, 2 * hp + hi, 0:DK + 1],
                        scs[t2][:, :, hi * 512:(hi + 1) * 512],
                        start=(t2 == 0), stop=(t2 == NT2 - 1), perf_mode=DR)

            for t2 in range(NT2):
                sc2 = p_c.tile([128, 2, 1024], FP8, tag="sc", bufs=3, name="sc")
                scs[t2] = sc2
                for tj in range(2):
                    t = 2 * t2 + tj
                    psW = scpsW.tile([128, 1024], F32, tag="scpsW", name="scpsW")
                    # cost^T preloads (full-row DR copies), then the two
                    # K=64 QK^T matmuls back-to-back: disjoint row groups
                    # (0:63 / 64:127) let them run concurrently on the PE.
                    for hi in range(2):
                        nc.tensor.matmul(psW[:, hi * 512:(hi + 1) * 512],
                                         idz, costT[t],
                                         start=True, stop=False, perf_mode=DR)
                    for hi in range(2):
                        nc.tensor.matmul(
                            psW[:, hi * 512:(hi + 1) * 512],
                            KT[hp][hi * 64:(hi + 1) * 64, t * 128:(t + 1) * 128],
                            QT[hp][hi * 64:(hi + 1) * 64, c * 512:(c + 1) * 512],
                            start=False, stop=True)
                    nc.scalar.activation(out=sc2[:, tj, :], in_=psW, func=AF.Exp,
                                         bias=ebias_t)
                # ctx lags one t-pair so the PE never heads-of-line blocks
                # on the exp of the pair it just produced.
                if t2 >= 1:
                    ctx_mms(t2 - 1)
            ctx_mms(NT2 - 1)
            for hi in range(2):
                # rowsum sits on PSUM partition 64; reciprocal lane-aligned,
                # then broadcast across partitions via a K=1 ones-matmul.
                # The x32 ctx fp8 scale rides on the ones stationary.
                rsb = p_c.tile([65, 512], F32R, tag="rsb", bufs=4, name="rsb")
                nc.vector.reciprocal(out=rsb[64:65, :], in_=cps[hi][DK:DK + 1, :])
                bps2 = ctxps.tile([64, 512], F32, tag="ctxps", name="bcps")
                nc.tensor.matmul(bps2, ones_t[64:65, :], rsb[64:65, :],
                                 start=True, stop=True)
                # stage through SBUF: a DVE op may not read two PSUM operands
                bc = p_c.tile([64, 512], F32, tag="bc", bufs=2, name="bc")
                nc.vector.tensor_copy(bc, bps2)
                nc.vector.tensor_tensor(
                    out=ctxT2[hp // 2][hi * 64:(hi + 1) * 64, hp % 2,
                                       c * 512:(c + 1) * 512],
                    in0=cps[hi][0:DK, :], in1=bc, op=ALU.mult)

            if c == 1 and hp == 1:
                # weave the c2=0 half of stage D into attention c=1 as ONE
                # cluster (a single Sqrt-table round trip on ACT) through
                # the spare PSUM bank (dps), so FFN1(c2=0) is ready the
                # moment attention ends.
                for st4 in range(4):
                    d_stage(st4, dps)
        if c == 1:
            aot_quads(0, dps)

    p_c.release()
    p_qkv.release()
    p_cost.release()
    ctxps.release()
    scpsW.release()
    dps.release()

    # ============ stages D (c2=1) + E, interleaved by c-half ============
    fcps = tc.alloc_tile_pool(name="fcps", bufs=2, space="PSUM", side="right")
    tps2 = tc.alloc_tile_pool(name="tps2", bufs=1, space="PSUM", side="right")
    p_e = tc.alloc_tile_pool(name="p_e", bufs=2, side="right")
    p_etmp = tc.alloc_tile_pool(name="p_etmp", bufs=3, side="right")
    f1ps = tc.alloc_tile_pool(name="f1ps", bufs=3, space="PSUM", side="right")
    f2ps = tc.alloc_tile_pool(name="f2ps", bufs=2, space="PSUM", side="right")

    def ffn1(c2, weave=None):
        h1T = [p_e.tile([128, 512], BF16, tag=f"h1t{jt}", name=f"h1t{jt}")
               for jt in range(FT)]
        for jt in range(FT):
            ps = f1ps.tile([128, 512], F32, tag="f1ps", name="f1ps")
            for d in range(DT):
                nc.tensor.matmul(ps, w1T[d][:, jt * 128:(jt + 1) * 128],
                                 aoT[d][:, c2 * 512:(c2 + 1) * 512],
                                 start=(d == 0), stop=(d == DT - 1))
            nc.scalar.activation(out=h1T[jt], in_=ps, func=AF.Relu,
                                 bias=b1_t[jt], scale=1.0)
            if weave is not None and jt % 4 == 3:
                weave(jt // 4)
        return h1T

    def ffn2(c2, h1T):
        for sti in range(4):
            st = c2 * 4 + sti
            ps = f2ps.tile([128, 512], F32, tag="f2ps", name="f2ps")
            for jt in range(FT):
                nc.tensor.matmul(ps, h1T[jt][:, sti * 128:(sti + 1) * 128],
                                 w2T[jt], start=(jt == 0), stop=(jt == FT - 1))
            f = p_etmp.tile([128, Dm], F32, tag="ffn_f", name="ffn_f")
            nc.vector.tensor_tensor(out=f, in0=ps, in1=attn_out[st], op=ALU.add)
            nc.gpsimd.tensor_add(f, f, b2_b)
            y = p_etmp.tile([128, Dm], F32, tag="ffn_y", name="ffn_y")
            layer_norm(f, y, ln2g_b, ln2b_b, p_etmp)
            nc.sync.dma_start(out=out_ap[st * 128:(st + 1) * 128, :], in_=y)

    # FFN(c2=0) runs on the PE while the c2=1 LN chains drain on DVE/Pool
    # (one fc tile woven per 4 FFN1 groups); aoT(g=1) transposes land
    # between FFN1(0) and FFN2(0).
    h1T_0 = ffn1(0, weave=lambda k: d_stage(4 + k, fcps))
    ffn2(0, h1T_0)
    aot_quads(1, tps2)
    h1T_1 = ffn1(1)
    ffn2(1, h1T_1)

    # release everything, LIFO per side
    f2ps.release()
    f1ps.release()
    p_etmp.release()
    p_e.release()
    tps2.release()
    fcps.release()
    p_dtmp.release()
    p_x.release()
    p_ctx.release()
    p_fcw.release()
    # left side
    p_d.release()
    p_w2.release()
    p_w1.release()
    singles.release()


def build_nc():
    from concourse import bacc

    nc = bacc.Bacc("TRN2", target_bir_lowering=False, debug=False)
    io = {
        name: nc.dram_tensor(name, list(shape), F32, kind="ExternalInput").ap()
        for name, shape in INPUT_SHAPES.items()
    }
    out_ap = nc.dram_tensor("out", [S, Dm], F32, kind="ExternalOutput").ap()
    with tile.TileContext(nc) as tc:
        _build(tc, io, out_ap)
    nc.compile()
    return nc


_NC_CACHE = None


def get_nc():
    global _NC_CACHE
    if _NC_CACHE is None:
        _NC_CACHE = build_nc()
    return _NC_CACHE


def kernel(**inputs):
    from concourse.bass_utils import run_bass_kernel_spmd

    nc = get_nc()
    in_maps = []
    for b in range(NCORES):
        m = {}
        for name in INPUT_SHAPES:
            arr = np.ascontiguousarray(inputs[name], dtype=np.float32)
            if name in ("enc_input", "cost_mat"):
                arr = np.ascontiguousarray(arr[b])
            m[name] = arr
        in_maps.append(m)
    res = run_bass_kernel_spmd(nc, in_maps, core_ids=list(range(NCORES)))
    return np.stack([res.results[b]["out"] for b in range(NCORES)], axis=0)



# revision 54
# speedup vs baseline: 13.2746x; 13.2746x over previous
"""Trainium2 Bass kernel for a MixEncoderLayer (attention w/ additive cost
matrix bias + FFN), batch 8, seq 1024, d_model 512, 8 heads, d_ff 2048.

Strategy: pure data parallelism — one batch element per NeuronCore, 8 cores,
no collectives.  Inside each core:

  X^T / W^T built via PE transposes; all matmuls in float32r (full-rate
  fp32 streaming on the PE at N=512).

  Attention is computed in "key-major" layout: scores^T[k, q], so softmax
  weights come out in exactly the layout needed as the moving operand of
  the attn@V matmul (no transposes of the 8.4M-element attention matrix).
  The cost-matrix bias is preloaded into PSUM with an identity-matmul
  (PE moves elements ~2.5x faster than DVE), the QK^T matmul accumulates
  on top, and the ACT engine applies exp directly from a wide 4-bank PSUM
  tile (amortizing its 352-cycle fixed overhead).  Softmax skips max
  subtraction (scores are O(+-6), exp is safe in f32); row sums come from
  augmenting V with a ones column ([V_h | 1], M=65) so ctx^T and rowsum^T
  fall out of one PSUM accumulation group; normalization is a reciprocal +
  ones-matmul partition-broadcast multiply on ctx^T (64x1024 per head)
  instead of on the 1024x1024 attention matrix.

Pools are stack-allocated per side; left = long-lived (released at end,
LIFO), right = stage-scoped.
"""

import numpy as np

import concourse.bass as bass
import concourse.mybir as mybir
import concourse.tile as tile
from concourse.masks import make_identity

F32 = mybir.dt.float32
F32R = mybir.dt.float32r
FP8 = mybir.dt.float8e4
BF16 = mybir.dt.bfloat16
AF = mybir.ActivationFunctionType
ALU = mybir.AluOpType
DR = mybir.MatmulPerfMode.DoubleRow

S, Dm, H, DK, DF = 1024, 512, 8, 64, 2048
ST, DT, FT = S // 128, Dm // 128, DF // 128  # 8, 4, 16
NCORES = 8
LN_EPS = 1e-6
INV_SQRT_DK = 0.125  # 1/sqrt(64)

INPUT_SHAPES = {
    "enc_input": (S, Dm),
    "cost_mat": (S, S),
    "wq": (Dm, Dm),
    "wk": (Dm, Dm),
    "wv": (Dm, Dm),
    "fc_w": (Dm, Dm),
    "ln1_g": (Dm,),
    "ln1_b": (Dm,),
    "w1": (DF, Dm),
    "b1": (DF,),
    "w2": (Dm, DF),
    "b2": (Dm,),
    "ln2_g": (Dm,),
    "ln2_b": (Dm,),
}


def _build(tc, io, out_ap):
    nc = tc.nc
    with nc.allow_low_precision(reason="f32r matmul operands; accumulation stays f32 in PSUM"):
        _build_inner(tc, io, out_ap)


def _build_inner(tc, io, out_ap):
    nc = tc.nc
    ev_cnt = [0]

    def evict_copy(dst, src, scale=None, eng="v"):
        """PSUM -> SBUF copy.  eng='v': DVE; eng='s': ACT (used for stage-A
        evictions where the ACT engine is otherwise idle); eng='a' alternates
        DVE/ACT so neither queue serializes stage A."""
        ev_cnt[0] += 1
        if eng == "a":
            eng = "s" if ev_cnt[0] % 2 else "v"
        if eng == "s":
            if scale is None:
                nc.scalar.copy(dst, src)
            else:
                nc.scalar.mul(dst, src, scale)
        elif scale is None:
            nc.vector.tensor_copy(dst, src)
        else:
            nc.vector.tensor_scalar_mul(out=dst, in0=src, scalar1=scale)

    # ---------------- long-lived pools (left stack) ----------------
    singles = tc.alloc_tile_pool(name="singles", bufs=1, side="left")
    # right-stack pools that live A/C -> D (bottom of the right stack)
    p_fcw = tc.alloc_tile_pool(name="p_fcw", bufs=1, side="right")
    p_ctx = tc.alloc_tile_pool(name="p_ctx", bufs=1, side="right")

    ident = singles.tile([128, 128], F32, tag="ident")
    make_identity(nc, ident)
    identR = singles.tile([128, 128], F32R, tag="identR")
    nc.vector.tensor_copy(identR, ident)
    # fp8 [I|0] / [0|I] stationaries: DoubleRow "copy" of one 512-col half of
    # a [128, 2, 512] cost tile into PSUM at 2 cols/cycle (2x the f32r
    # identity-matmul preload).
    idzA = singles.tile([128, 2, 128], FP8, tag="idzA")
    nc.gpsimd.memset(idzA, 0.0)
    nc.vector.tensor_copy(idzA[:, 0, :], ident)
    idzB = singles.tile([128, 2, 128], FP8, tag="idzB")
    nc.gpsimd.memset(idzB, 0.0)
    nc.vector.tensor_copy(idzB[:, 1, :], ident)
    eps_t = singles.tile([128, 1], F32, tag="eps")
    nc.gpsimd.memset(eps_t, LN_EPS)
    ones_f32 = singles.tile([128, 1], F32, tag="ones_f32")
    nc.vector.memset(ones_f32, 1.0)
    ones8_f32 = singles.tile([128, 1], F32, tag="ones8_f32")
    nc.vector.memset(ones8_f32, 8.0)
    c32_f32 = singles.tile([128, 1], F32, tag="c32_f32")
    nc.vector.memset(c32_f32, 32.0)
    # rowsum-broadcast stationary carries the x32 ctx fp8 scale
    ones_t = singles.tile([128, 64], F32R, tag="ones")
    nc.vector.tensor_copy(ones_t, c32_f32.to_broadcast((128, 64)))
    fcrs_t = singles.tile([128, 1], F32, tag="fcrs")
    nc.vector.memset(fcrs_t, 1.0 / (32.0 * 32.0))
    # exp(score - 1.5): keeps exp outputs under fp8e4m3's 448 max for the
    # score tails (cancels exactly in the softmax normalization)
    ebias_t = singles.tile([128, 1], F32, tag="ebias")
    nc.vector.memset(ebias_t, -1.5)

    def layer_norm(src, dst, g_b, b_b, pool, gb_eng=None, stt_eng=None,
                   newton=False):
        """dst = LN(src) * g + b over free dim (512).  gb_eng picks which
        engine applies g/b: DVE when the result feeds the PE (LN1 -> aoT
        transposes), the idle-but-slow gpsimd when it only feeds DMA (LN2).
        newton=True computes istd with a DVE-only Newton rsqrt (var is ~1
        for the residual-stream tiles) so no Sqrt lands in the ACT queue
        mid-attention, where it would thrash the Exp table."""
        gb = gb_eng if gb_eng is not None else nc.gpsimd
        stats = pool.tile([128, 6], F32, tag="ln_stats", bufs=3, name="ln_stats")
        mv = pool.tile([128, 2], F32, tag="ln_mv", bufs=3, name="ln_mv")
        nc.vector.bn_stats(out=stats, in_=src)
        nc.vector.bn_aggr(out=mv, in_=stats)
        istd = pool.tile([128, 1], F32, tag="ln_istd", bufs=3, name="ln_istd")
        if newton:
            v = mv[:, 1:2]
            y = istd
            nc.vector.tensor_scalar(out=y, in0=v, scalar1=-0.5, op0=ALU.mult,
                                    scalar2=1.5 - 0.5 * LN_EPS, op1=ALU.add)
            s = pool.tile([128, 1], F32, tag="ln_nt", bufs=2, name="ln_nt")
            for _ in range(3):
                nc.vector.tensor_tensor(out=s, in0=y, in1=y, op=ALU.mult)
                nc.vector.tensor_tensor(out=s, in0=s, in1=v, op=ALU.mult)
                nc.vector.tensor_scalar(out=s, in0=s, scalar1=-0.5,
                                        op0=ALU.mult, scalar2=1.5, op1=ALU.add)
                nc.vector.tensor_tensor(out=y, in0=y, in1=s, op=ALU.mult)
        else:
            nc.scalar.activation(out=istd, in_=mv[:, 1:2], func=AF.Sqrt,
                                 bias=eps_t)
            nc.vector.reciprocal(out=istd, in_=istd)
        xn = pool.tile([128, Dm], F32, tag="ln_xn", bufs=2, name="ln_xn")
        (stt_eng if stt_eng is not None else nc.vector).scalar_tensor_tensor(
            out=xn, in0=src, scalar=mv[:, 0:1], in1=istd.to_broadcast((128, Dm)),
            op0=ALU.subtract, op1=ALU.mult)
        gb.tensor_mul(dst, xn, g_b)
        gb.tensor_add(dst, dst, b_b)

    # ================= stage A: loads + transposes =================
    p_x = tc.alloc_tile_pool(name="p_x", bufs=1, side="right")      # A -> D
    p_dtmp = tc.alloc_tile_pool(name="p_dtmp", bufs=2, side="right")  # D scratch
    p_cost = tc.alloc_tile_pool(name="p_cost", bufs=1, side="right")  # A -> C
    p_qkv = tc.alloc_tile_pool(name="p_qkv", bufs=1, side="right")  # B -> C
    p_ab = tc.alloc_tile_pool(name="p_ab", bufs=1, side="right")    # A -> B
    p_stgA = tc.alloc_tile_pool(name="p_stgA", bufs=5, side="right")  # A only
    tps = tc.alloc_tile_pool(name="tps", bufs=4, space="PSUM", side="right")
    bps = tc.alloc_tile_pool(name="bps", bufs=3, space="PSUM", side="right")

    def transpose_quad(psum_pool, psum_tag, dst_wide, srcs, scale=None, eng="v"):
        """Transpose up to 4 [128,128] blocks into one PSUM bank, evict once.
        f32r-mode (1.5 cyc/row vs 2 for f32) when the source tile is f32r."""
        n = len(srcs)
        ps = psum_pool.tile([128, n * 128], F32, tag=psum_tag, name=psum_tag)
        r = srcs[0].dtype == F32R
        idt = identR if r else ident
        for i, s in enumerate(srcs):
            sl = ps[:, i * 128:(i + 1) * 128]
            nc.tensor.transpose(sl.bitcast(F32R) if r else sl, s, idt)
        evict_copy(dst_wide, ps, scale=scale, eng=eng)

    # X + X^T
    xsb = []
    for st in range(ST):
        t = p_x.tile([128, Dm], F32R, tag=f"x{st}", name=f"x{st}")
        nc.sync.dma_start(
            out=t,
            in_=io["enc_input"][st * 128:(st + 1) * 128, :].bitcast(F32R))
        xsb.append(t)
    # X^T, W^T in fp8 with the d-tile PAIR in dim1 (DoubleRow contraction):
    # QKV/fc projections then take 2 DR matmuls instead of 4 f32r ones.
    # Weights are pre-scaled x32 into fp8 normal range (std 0.02 -> 0.64);
    # the x32 is divided back out at each PSUM eviction.
    WS = 32.0
    XT2 = [p_ab.tile([128, 2, S], FP8, tag=f"xt{dp}", name=f"xt{dp}")
           for dp in range(DT // 2)]
    XT = [XT2[d // 2][:, d % 2, :] for d in range(DT)]
    for d in range(DT):
        for g in range(ST // 4):
            transpose_quad(
                tps, "tps", XT[d][:, g * 512:(g + 1) * 512],
                [xsb[g * 4 + i][:, d * 128:(d + 1) * 128] for i in range(4)],
                eng="a")

    def load_transposed(stg_pool, psum_pool, psum_tag, wap, dst_tiles, stg_tag,
                        group=4, scale=None, eng="v", dma="sync"):
        """wap: DRAM [nout, nin]; dst_tiles[k]: [128, nout] covering nin rows."""
        nout, nin = wap.shape
        nit = nout // 128
        dma_eng = nc.sync if dma == "sync" else nc.gpsimd
        for g in range(0, nit, group):
            n = min(group, nit - g)
            stgs = []
            for i in range(n):
                stg = stg_pool.tile([128, nin], F32R, tag=stg_tag, name=stg_tag)
                dma_eng.dma_start(
                    out=stg,
                    in_=wap[(g + i) * 128:(g + i + 1) * 128, :].bitcast(F32R))
                stgs.append(stg)
            for dt_ in range(nin // 128):
                transpose_quad(
                    psum_pool, psum_tag,
                    dst_tiles[dt_][:, g * 128:(g + n) * 128],
                    [stgs[i][:, dt_ * 128:(dt_ + 1) * 128] for i in range(n)],
                    scale=scale, eng=eng)

    wqT2 = [p_ab.tile([128, 2, Dm], FP8, tag=f"wqt{dp}", name=f"wqt{dp}")
            for dp in range(DT // 2)]
    wkT2 = [p_ab.tile([128, 2, Dm], FP8, tag=f"wkt{dp}", name=f"wkt{dp}")
            for dp in range(DT // 2)]
    wvT2 = [p_ab.tile([128, 2, Dm], FP8, tag=f"wvt{dp}", name=f"wvt{dp}")
            for dp in range(DT // 2)]
    fcwT2 = [p_fcw.tile([128, 2, Dm], FP8, tag=f"fcwt{dp}", name=f"fcwt{dp}")
             for dp in range(DT // 2)]
    wqT = [wqT2[d // 2][:, d % 2, :] for d in range(DT)]
    wkT = [wkT2[d // 2][:, d % 2, :] for d in range(DT)]
    wvT = [wvT2[d // 2][:, d % 2, :] for d in range(DT)]
    fcwT = [fcwT2[d // 2][:, d % 2, :] for d in range(DT)]
    load_transposed(p_stgA, tps, "tps", io["wq"], wqT, "stg512", scale=WS, eng="a")
    load_transposed(p_stgA, tps, "tps", io["wk"], wkT, "stg512", scale=WS, eng="a")
    load_transposed(p_stgA, tps, "tps", io["wv"], wvT, "stg512", scale=WS, eng="a", dma="pool")

    # small constants on the Pool-engine DMA queue so they don't delay the
    # big input loads on the SP queue or the ACT-engine evictions
    def bcast_row(name, src1d):  # [Dm] dram -> [128, Dm] sbuf (partition bcast)
        t = singles.tile([128, Dm], F32, tag=name, name=name)
        nc.scalar.dma_start(out=t, in_=src1d[None, :].to_broadcast((128, Dm)))
        return t

    ln1g_b = bcast_row("ln1g_b", io["ln1_g"])
    ln1b_b = bcast_row("ln1b_b", io["ln1_b"])
    ln2g_b = bcast_row("ln2g_b", io["ln2_g"])
    ln2b_b = bcast_row("ln2b_b", io["ln2_b"])
    b2_b = bcast_row("b2_b", io["b2"])
    b1_t = []
    for jt in range(FT):
        t = singles.tile([128, 1], F32, tag=f"b1_{jt}", name=f"b1_{jt}")
        nc.scalar.dma_start(out=t, in_=io["b1"][jt * 128:(jt + 1) * 128][:, None])
        b1_t.append(t)

    # cost^T in fp8 as [128, 2, 512] (the two 512-col q-halves are the
    # DoubleRow k-pair slots; idzA/idzB select which half lands in PSUM).
    costT = [p_cost.tile([128, 2, 512], FP8, tag=f"ct{k}", name=f"ct{k}")
             for k in range(ST)]
    costT_2d = [ct[:, :, :].rearrange("p a b -> p (a b)") for ct in costT]
    load_transposed(p_stgA, tps, "tps", io["cost_mat"], costT_2d, "stg1024", eng="a", dma="pool")
    load_transposed(p_stgA, tps, "tps", io["fc_w"], fcwT, "stg512", scale=WS, eng="a", dma="pool")

    # ================= stage B: QKV projections =================
    QT = [p_qkv.tile([128, S], F32R, tag=f"qt{i}", name=f"qt{i}") for i in range(DT)]
    KT = [p_qkv.tile([128, S], F32R, tag=f"kt{i}", name=f"kt{i}") for i in range(DT)]
    # V (x8, fp8) + ones(=8) column, t-pairs in dim1 for the DoubleRow ctx
    # matmul; per-head stride 80B keeps every AP step 16B-aligned.  The x8
    # scale cancels in the rowsum normalization.
    VP = 80
    vaug = [p_qkv.tile([128, 2, H, VP], FP8, tag=f"va{t2}", name=f"va{t2}")
            for t2 in range(ST // 2)]

    for it in range(DT):
        for c in range(2):
            ps = bps.tile([128, 512], F32, tag="bps", name="bps")
            for p in range(DT // 2):
                nc.tensor.matmul(ps, wqT2[p][:, :, it * 128:(it + 1) * 128],
                                 XT2[p][:, :, c * 512:(c + 1) * 512],
                                 start=(p == 0), stop=(p == DT // 2 - 1),
                                 perf_mode=DR)
            # fold 1/sqrt(dk) and the 1/32 weight-scale into Q
            nc.vector.tensor_scalar_mul(
                out=QT[it][:, c * 512:(c + 1) * 512], in0=ps,
                scalar1=INV_SQRT_DK / WS)
    for it in range(DT):
        for c in range(2):
            ps = bps.tile([128, 512], F32, tag="bps", name="bps")
            for p in range(DT // 2):
                nc.tensor.matmul(ps, wkT2[p][:, :, it * 128:(it + 1) * 128],
                                 XT2[p][:, :, c * 512:(c + 1) * 512],
                                 start=(p == 0), stop=(p == DT // 2 - 1),
                                 perf_mode=DR)
            nc.scalar.mul(KT[it][:, c * 512:(c + 1) * 512], ps, 1.0 / WS)
    for st in range(ST):
        t2, tj = divmod(st, 2)
        nc.vector.tensor_copy(
            out=vaug[t2][:, tj, :, DK:DK + 1].rearrange("p h o -> p (h o)"),
            in_=ones8_f32.to_broadcast((128, H)))
        ps = bps.tile([128, 512], F32, tag="bps", name="bps")
        for p in range(DT // 2):
            nc.tensor.matmul(ps, XT2[p][:, :, st * 128:(st + 1) * 128], wvT2[p],
                             start=(p == 0), stop=(p == DT // 2 - 1),
                             perf_mode=DR)
        nc.vector.tensor_scalar_mul(
            out=vaug[t2][:, tj, :, 0:DK],
            in0=ps.rearrange("p (h e) -> p h e", h=H), scalar1=8.0 / WS)

    p_stgA.release()
    p_ab.release()
    bps.release()
    tps.release()

    # ================= stage C: attention (key-major) =================
    # ctx^T in fp8 (x32) with d-tile pairs in dim1 for the DR fc matmul
    ctxT2 = [p_ctx.tile([128, 2, S], FP8, tag=f"cx{dp}", name=f"cx{dp}")
             for dp in range(DT // 2)]
    p_c = tc.alloc_tile_pool(name="p_c", bufs=2, side="right")
    scpsW = tc.alloc_tile_pool(name="scpsW", bufs=2, space="PSUM", side="right")
    ctxps = tc.alloc_tile_pool(name="ctxps", bufs=3, space="PSUM", side="right")

    # w1 + w2 load+transpose traced mid-C: fills PE gaps during the
    # ACT-bound attention phase (left-side space, no deps on stage-C pools).
    p_w1 = tc.alloc_tile_pool(name="p_w1", bufs=1, side="left")
    p_stgW1 = tc.alloc_tile_pool(name="p_stgW1", bufs=5, side="left")
    tpsW = tc.alloc_tile_pool(name="tpsW", bufs=1, space="PSUM", side="left")
    w1T = [p_w1.tile([128, DF], BF16, tag=f"w1t{d}", name=f"w1t{d}")
           for d in range(DT)]
    load_transposed(p_stgW1, tpsW, "tpsW", io["w1"], w1T, "stgw1")
    p_stgW1.release()
    p_w2 = tc.alloc_tile_pool(name="p_w2", bufs=1, side="left")
    p_stgW2 = tc.alloc_tile_pool(name="p_stgW2", bufs=3, side="left")
    w2T = [p_w2.tile([128, Dm], BF16, tag=f"w2t{j}", name=f"w2t{j}")
           for j in range(FT)]
    load_transposed(p_stgW2, tpsW, "tpsW", io["w2"], w2T, "stgw2", group=2,
                    dma="pool")
    p_stgW2.release()
    tpsW.release()
    # 1-bank PSUM pool + stage-D SBUF, so the c2=0 half of fc/LN1/aoT can
    # run woven into attention c=1 (all 7 right-side banks are then held by
    # the attention pools).
    dps = tc.alloc_tile_pool(name="dps", bufs=1, space="PSUM", side="left")
    p_d = tc.alloc_tile_pool(name="p_d", bufs=1, side="left")  # D -> E
    attn_out = [p_d.tile([128, Dm], F32, tag=f"ao{st}", name=f"ao{st}")
                for st in range(ST)]
    aoT = [p_d.tile([128, S], BF16, tag=f"aot{d}", name=f"aot{d}") for d in range(DT)]

    def d_stage(st, psum_pool):
        """fc + residual + LN1 for one token tile."""
        ps = psum_pool.tile([128, 512], F32, tag="fcps", name="fcps")
        for p in range(DT // 2):
            nc.tensor.matmul(ps, ctxT2[p][:, :, st * 128:(st + 1) * 128],
                             fcwT2[p], start=(p == 0), stop=(p == DT // 2 - 1),
                             perf_mode=DR)
        a = p_dtmp.tile([128, Dm], F32, tag="attnin", name="attnin")
        # undo the x32 ctx and x32 fc_w fp8 scales, then add the residual
        nc.vector.scalar_tensor_tensor(
            out=a, in0=ps, scalar=fcrs_t, in1=xsb[st],
            op0=ALU.mult, op1=ALU.add)
        layer_norm(a, attn_out[st], ln1g_b, ln1b_b, p_dtmp,
                   newton=(psum_pool is dps))

    def aot_quads(g, psum_pool):
        for d in range(DT):
            transpose_quad(
                psum_pool, "fcps", aoT[d][:, g * 512:(g + 1) * 512],
                [attn_out[g * 4 + i][:, d * 128:(d + 1) * 128]
                 for i in range(4)])

    # c outer so the fc matmuls of stage D (which need all heads but only one
    # 512-token half) can start when attention is half done.
    NT2 = ST // 2
    for c in range(2):
        idz = idzA if c == 0 else idzB
        for hp in range(H // 2):
            cps = [ctxps.tile([DK + 1, 512], F32, tag="ctxps", name="ctxps")
                   for _ in range(2)]
            scs = [None] * NT2

            def ctx_mms(t2):
                for hi in range(2):
                    nc.tensor.matmul(
                        cps[hi], vaug[t2][:, :, 2 * hp + hi, 0:DK + 1],
                        scs[t2][:, :, hi * 512:(hi + 1) * 512],
                        start=(t2 == 0), stop=(t2 == NT2 - 1), perf_mode=DR)

            for t2 in range(NT2):
                sc2 = p_c.tile([128, 2, 1024], FP8, tag="sc", bufs=3, name="sc")
                scs[t2] = sc2
                for tj in range(2):
                    t = 2 * t2 + tj
                    psW = scpsW.tile([128, 1024], F32, tag="scpsW", name="scpsW")
                    # cost^T preloads (full-row DR copies), then the two
                    # K=64 QK^T matmuls back-to-back: disjoint row groups
                    # (0:63 / 64:127) let them run concurrently on the PE.
                    for hi in range(2):
                        nc.tensor.matmul(psW[:, hi * 512:(hi + 1) * 512],
                                         idz, costT[t],
                                         start=True, stop=False, perf_mode=DR)
                    for hi in range(2):
                        nc.tensor.matmul(
                            psW[:, hi * 512:(hi + 1) * 512],
                            KT[hp][hi * 64:(hi + 1) * 64, t * 128:(t + 1) * 128],
                            QT[hp][hi * 64:(hi + 1) * 64, c * 512:(c + 1) * 512],
                            start=False, stop=True)
                    nc.scalar.activation(out=sc2[:, tj, :], in_=psW, func=AF.Exp,
                                         bias=ebias_t)
                # ctx lags one t-pair so the PE never heads-of-line blocks
                # on the exp of the pair it just produced.
                if t2 >= 1:
                    ctx_mms(t2 - 1)
            ctx_mms(NT2 - 1)
            for hi in range(2):
                # rowsum sits on PSUM partition 64; reciprocal lane-aligned,
                # then broadcast across partitions via a K=1 ones-matmul.
                # The x32 ctx fp8 scale rides on the ones stationary.
                rsb = p_c.tile([65, 512], F32R, tag="rsb", bufs=4, name="rsb")
                nc.vector.reciprocal(out=rsb[64:65, :], in_=cps[hi][DK:DK + 1, :])
                bps2 = ctxps.tile([64, 512], F32, tag="ctxps", name="bcps")
                nc.tensor.matmul(bps2, ones_t[64:65, :], rsb[64:65, :],
                                 start=True, stop=True)
                # stage through SBUF: a DVE op may not read two PSUM operands
                bc = p_c.tile([64, 512], F32, tag="bc", bufs=2, name="bc")
                nc.vector.tensor_copy(bc, bps2)
                nc.vector.tensor_tensor(
                    out=ctxT2[hp // 2][hi * 64:(hi + 1) * 64, hp % 2,
                                       c * 512:(c + 1) * 512],
                    in0=cps[hi][0:DK, :], in1=bc, op=ALU.mult)

            if c == 1 and hp == 0:
                # weave the c2=0 half of stage D into attention c=1 as ONE
                # cluster (a single Sqrt-table round trip on ACT) through
                # the spare PSUM bank (dps), so FFN1(c2=0) is ready the
                # moment attention ends.
                for st4 in range(4):
                    d_stage(st4, dps)
            if c == 1 and hp == 2:
                aot_quads(0, dps)

    p_c.release()
    p_qkv.release()
    p_cost.release()
    ctxps.release()
    scpsW.release()
    dps.release()

    # ============ stages D (c2=1) + E, interleaved by c-half ============
    fcps = tc.alloc_tile_pool(name="fcps", bufs=2, space="PSUM", side="right")
    tps2 = tc.alloc_tile_pool(name="tps2", bufs=1, space="PSUM", side="right")
    p_e = tc.alloc_tile_pool(name="p_e", bufs=2, side="right")
    p_etmp = tc.alloc_tile_pool(name="p_etmp", bufs=3, side="right")
    f1ps = tc.alloc_tile_pool(name="f1ps", bufs=3, space="PSUM", side="right")
    f2ps = tc.alloc_tile_pool(name="f2ps", bufs=2, space="PSUM", side="right")

    def ffn1(c2, weave=None):
        h1T = [p_e.tile([128, 512], BF16, tag=f"h1t{jt}", name=f"h1t{jt}")
               for jt in range(FT)]
        for jt in range(FT):
            ps = f1ps.tile([128, 512], F32, tag="f1ps", name="f1ps")
            for d in range(DT):
                nc.tensor.matmul(ps, w1T[d][:, jt * 128:(jt + 1) * 128],
                                 aoT[d][:, c2 * 512:(c2 + 1) * 512],
                                 start=(d == 0), stop=(d == DT - 1))
            nc.scalar.activation(out=h1T[jt], in_=ps, func=AF.Relu,
                                 bias=b1_t[jt], scale=1.0)
            if weave is not None and jt % 4 == 3:
                weave(jt // 4)
        return h1T

    def ffn2(c2, h1T):
        for sti in range(4):
            st = c2 * 4 + sti
            ps = f2ps.tile([128, 512], F32, tag="f2ps", name="f2ps")
            for jt in range(FT):
                nc.tensor.matmul(ps, h1T[jt][:, sti * 128:(sti + 1) * 128],
                                 w2T[jt], start=(jt == 0), stop=(jt == FT - 1))
            f = p_etmp.tile([128, Dm], F32, tag="ffn_f", name="ffn_f")
            nc.vector.tensor_tensor(out=f, in0=ps, in1=attn_out[st], op=ALU.add)
            nc.gpsimd.tensor_add(f, f, b2_b)
            y = p_etmp.tile([128, Dm], F32, tag="ffn_y", name="ffn_y")
            layer_norm(f, y, ln2g_b, ln2b_b, p_etmp)
            nc.sync.dma_start(out=out_ap[st * 128:(st + 1) * 128, :], in_=y)

    # FFN(c2=0) runs on the PE while the c2=1 LN chains drain on DVE/Pool
    # (one fc tile woven per 4 FFN1 groups); aoT(g=1) transposes land
    # between FFN1(0) and FFN2(0).
    h1T_0 = ffn1(0, weave=lambda k: d_stage(4 + k, fcps))
    ffn2(0, h1T_0)
    aot_quads(1, tps2)
    h1T_1 = ffn1(1)
    ffn2(1, h1T_1)

    # release everything, LIFO per side
    f2ps.release()
    f1ps.release()
    p_etmp.release()
    p_e.release()
    tps2.release()
    fcps.release()
    p_dtmp.release()
    p_x.release()
    p_ctx.release()
    p_fcw.release()
    # left side
    p_d.release()
    p_w2.release()
    p_w1.release()
    singles.release()


def build_nc(unroll=1):
    from concourse import bacc

    nc = bacc.Bacc("TRN2", target_bir_lowering=False, debug=False)
    io = {
        name: nc.dram_tensor(name, list(shape), F32, kind="ExternalInput").ap()
        for name, shape in INPUT_SHAPES.items()
    }
    out_ap = nc.dram_tensor("out", [S, Dm], F32, kind="ExternalOutput").ap()
    with tile.TileContext(nc) as tc:
        for _ in range(unroll):
            _build(tc, io, out_ap)
    nc.compile()
    return nc


_NC_CACHE = None


def get_nc():
    global _NC_CACHE
    if _NC_CACHE is None:
        _NC_CACHE = build_nc()
    return _NC_CACHE


def kernel(**inputs):
    from concourse.bass_utils import run_bass_kernel_spmd

    nc = get_nc()
    in_maps = []
    for b in range(NCORES):
        m = {}
        for name in INPUT_SHAPES:
            arr = np.ascontiguousarray(inputs[name], dtype=np.float32)
            if name in ("enc_input", "cost_mat"):
                arr = np.ascontiguousarray(arr[b])
            m[name] = arr
        in_maps.append(m)
    res = run_bass_kernel_spmd(nc, in_maps, core_ids=list(range(NCORES)))
    return np.stack([res.results[b]["out"] for b in range(NCORES)], axis=0)

